# revision 1
# baseline (speedup 1.0000x reference)
"""SCAN-style bidirectional image-text similarity on 8 TRN2 NeuronCores.

Math (per text t, vs reference.compute_weiTexts):
  dots[l,nr] = tk_hat[l]. qn_hat[nr]         (hi/lo bf16 split, 3 matmuls)
  i2t:  leaky -> l2norm over r -> softmax/focal over l (partition dim, via
        ones-matmul column reduces) -> cosine(img_v, wei) where wei is never
        materialized:  w12 = sum_l e2*G,  |wei|^2 = e2^T K2 e2  (G = tv.img_v,
        K2 = tv gram).
  t2i:  leaky -> l2norm over l (ones-matmul) -> softmax/focal over r (free
        dim) -> cosine(tv, wei_img) via num = sum_r e2t*G and
        |wei_img|^2 = e2t^T V e2t with V = per-image img_v gram.

Sharding: 16 texts per core (8 pairs packed 2-per-128-partitions); images
replicated.  Each core computes its [n_img, 16] output columns; host concats.
NR axis processed in 4 quarters of 1536 cols to fit SBUF.
"""

import numpy as np
import ml_dtypes
from contextlib import ExitStack

import concourse.bass as bass
import concourse.bacc as bacc
import concourse.tile as tile
from concourse import mybir
from concourse.bass_utils import run_bass_kernel_spmd

BF = ml_dtypes.bfloat16
F32 = np.float32
EPS = F32(1e-8)

N, T, R, Lw, D = 128, 128, 48, 64, 1024
NR = N * R
CORES, TPC, PAIRS = 8, 16, 8
QT, QC, NQ, SL, JT = 4, 1536, 32, 384, 4   # quarters, cols/qt, n/qt, slice, slices/qt
CH = D // 128
LAM = 20.0

dt = mybir.dt
AX = mybir.AxisListType
OP = mybir.AluOpType
AF = mybir.ActivationFunctionType


def build_kernel(nc):
    def din(name, shape, d=dt.bfloat16):
        return nc.dram_tensor(name, shape, d, kind="ExternalInput").ap()

    qhi_d = din("qhi", [CH, 128, NR])
    qlo_d = din("qlo", [CH, 128, NR])
    ivt_d = din("ivt", [CH, 128, NR])
    rw1b_d = din("rw1b", [TPC, NR])
    vbd_d = din("vbd", [QT, 96, NQ // 2 * 96])
    tkhi_d = din("tkhi", [PAIRS, CH, 128, 128])
    tklo_d = din("tklo", [PAIRS, CH, 128, 128])
    tvt_d = din("tvt", [PAIRS, CH, 128, 128])
    k2bd_d = din("k2bd", [PAIRS, 128, 128])
    rw1t_d = din("rw1t", [PAIRS, 128, 1], dt.float32)
    expb_d = din("expb", [PAIRS, 128, 1], dt.float32)
    monw_d = din("monw", [PAIRS, 128, 1], dt.float32)
    invnw_d = din("invnw", [PAIRS, 2, 1], dt.float32)
    onesb_d = din("onesb", [128, 2])
    onesf_d = din("onesf", [2, 128], dt.float32)
    ident_d = din("ident", [128, 128])
    i2t_o = nc.dram_tensor("i2t_loc", [TPC, N], dt.float32, kind="ExternalOutput").ap()
    t2i_o = nc.dram_tensor("t2i_loc", [TPC, N], dt.float32, kind="ExternalOutput").ap()

    with tile.TileContext(nc) as tc, ExitStack() as ctx:
        cpool = ctx.enter_context(tc.tile_pool(name="const", bufs=1))
        bigq = ctx.enter_context(tc.tile_pool(name="bigq", bufs=1))
        pairc = ctx.enter_context(tc.tile_pool(name="pairc", bufs=2))
        wA = ctx.enter_context(tc.tile_pool(name="wA", bufs=2))
        wA1 = ctx.enter_context(tc.tile_pool(name="wA1", bufs=1))
        wB = ctx.enter_context(tc.tile_pool(name="wB", bufs=1))
        rowp = ctx.enter_context(tc.tile_pool(name="rowp", bufs=2))
        small = ctx.enter_context(tc.tile_pool(name="small", bufs=2))
        acc = ctx.enter_context(tc.tile_pool(name="acc", bufs=1))
        ps = ctx.enter_context(tc.tile_pool(name="ps", bufs=5, space="PSUM"))
        psr = ctx.enter_context(tc.tile_pool(name="psr", bufs=2, space="PSUM"))
        tpp = ctx.enter_context(tc.tile_pool(name="tpp", bufs=1, space="PSUM"))

        onesb = cpool.tile([128, 2], dt.bfloat16)
        nc.sync.dma_start(out=onesb, in_=onesb_d)
        onesf = cpool.tile([2, 128], dt.float32)
        nc.sync.dma_start(out=onesf, in_=onesf_d)
        ident = cpool.tile([128, 128], dt.bfloat16)
        nc.sync.dma_start(out=ident, in_=ident_d)
        b16 = cpool.tile([128, 1], dt.float32)
        nc.vector.memset(b16, 1e-16)
        b20 = cpool.tile([128, 1], dt.float32)
        nc.vector.memset(b20, 1e-20)

        SL2, JT2 = 512, 3          # psum-slice width for column ops

        for qt in range(QT):
            cols = slice(qt * QC, (qt + 1) * QC)
            qhi_t = bigq.tile([128, CH, QC], dt.bfloat16, tag="qhi")
            qlo_t = bigq.tile([128, CH, QC], dt.bfloat16, tag="qlo")
            ivt_t = bigq.tile([128, CH, QC], dt.bfloat16, tag="ivt")
            for ch in range(CH):
                nc.sync.dma_start(out=qhi_t[:, ch, :], in_=qhi_d[ch][:, cols])
                nc.sync.dma_start(out=qlo_t[:, ch, :], in_=qlo_d[ch][:, cols])
                nc.sync.dma_start(out=ivt_t[:, ch, :], in_=ivt_d[ch][:, cols])
            rw1b_t = bigq.tile([TPC, QC], dt.bfloat16, tag="rw1b")
            nc.sync.dma_start(out=rw1b_t, in_=rw1b_d[:, cols])
            v_bd = bigq.tile([96, NQ // 2, 96], dt.bfloat16, tag="vbd")
            nc.sync.dma_start(out=v_bd, in_=vbd_d[qt].rearrange("p (m c) -> p m c", c=96))

            w12_all = acc.tile([TPC, QC], dt.float32, tag="w12")
            w2sq_all = acc.tile([TPC, QC], dt.float32, tag="w2sq")

            for p in range(PAIRS):
                tkhi = pairc.tile([128, CH, 128], dt.bfloat16, tag="tkhi")
                nc.sync.dma_start(out=tkhi, in_=tkhi_d[p].rearrange("c p m -> p c m"))
                tklo = pairc.tile([128, CH, 128], dt.bfloat16, tag="tklo")
                nc.sync.dma_start(out=tklo, in_=tklo_d[p].rearrange("c p m -> p c m"))
                tvt = pairc.tile([128, CH, 128], dt.bfloat16, tag="tvt")
                nc.sync.dma_start(out=tvt, in_=tvt_d[p].rearrange("c p m -> p c m"))
                k2bd = pairc.tile([128, 128], dt.bfloat16, tag="k2bd")
                nc.sync.dma_start(out=k2bd, in_=k2bd_d[p])
                rw1t = pairc.tile([128, 1], dt.float32, tag="rw1t")
                nc.sync.dma_start(out=rw1t, in_=rw1t_d[p])
                expb = pairc.tile([128, 1], dt.float32, tag="expb")
                nc.sync.dma_start(out=expb, in_=expb_d[p])
                monw = pairc.tile([128, 1], dt.float32, tag="monw")
                nc.sync.dma_start(out=monw, in_=monw_d[p])
                invnw = pairc.tile([2, 1], dt.float32, tag="invnw")
                nc.sync.dma_start(out=invnw, in_=invnw_d[p])

                # ---- phase A: dots + G matmuls, leaky, G->bf16
                lr = wA.tile([128, QC], dt.float32, tag="lr")
                gb = wA.tile([128, QC], dt.bfloat16, tag="gb")
                for j in range(JT2):
                    sl_ = slice(j * SL2, (j + 1) * SL2)
                    dps = ps.tile([128, SL2], dt.float32, tag="ps")
                    for i3, (tk_, q_) in enumerate(((tkhi, qhi_t), (tkhi, qlo_t),
                                                    (tklo, qhi_t))):
                        for ch in range(CH):
                            nc.tensor.matmul(dps, tk_[:, ch, :], q_[:, ch, sl_],
                                             start=(i3 == 0 and ch == 0),
                                             stop=(i3 == 2 and ch == CH - 1))
                    nc.scalar.copy(lr[:, sl_], dps)
                    nc.vector.scalar_tensor_tensor(lr[:, sl_], lr[:, sl_], 0.1,
                                                   lr[:, sl_], op0=OP.mult,
                                                   op1=OP.max)
                    gps = ps.tile([128, SL2], dt.float32, tag="ps")
                    for ch in range(CH):
                        nc.tensor.matmul(gps, tvt[:, ch, :], ivt_t[:, ch, sl_],
                                         start=(ch == 0), stop=(ch == CH - 1))
                    nc.vector.tensor_copy(gb[:, sl_], gps)

                lr3 = lr.rearrange("p (n r) -> p n r", r=48)

                # ---- phase B: norms
                sq = wB.tile([128, QC], dt.float32, tag="sq")
                nc.vector.tensor_tensor(sq, lr, lr, op=OP.mult)
                sqh = wA1.tile([128, QC], dt.bfloat16, tag="hi")
                nc.vector.tensor_copy(sqh, sq)
                sql = wA1.tile([128, QC], dt.bfloat16, tag="lo")
                nc.vector.tensor_tensor(sql, sq, sqh, op=OP.subtract)
                segn = small.tile([128, NQ], dt.float32, tag="segn")
                nc.vector.tensor_reduce(segn, sq.rearrange("p (n r) -> p n r", r=48),
                                        axis=AX.X, op=OP.add)
                rs = small.tile([128, NQ], dt.float32, tag="rs")
                nc.scalar.activation(rs, segn, AF.Sqrt, bias=b16)
                nc.vector.reciprocal_approx_fast(out=rs, in_=rs)

                rows = rowp.tile([2, QC], dt.float32, tag="row")
                for j in range(JT2):
                    sl_ = slice(j * SL2, (j + 1) * SL2)
                    cps = psr.tile([2, SL2], dt.float32, tag="psr")
                    nc.tensor.matmul(cps, onesb, sqh[:, sl_], start=True, stop=False)
                    nc.tensor.matmul(cps, onesb, sql[:, sl_], start=False, stop=True)
                    nc.scalar.activation(rows[:, sl_], cps, AF.Sqrt, bias=b20[0:2])
                nc.vector.reciprocal_approx_fast(out=rows, in_=rows)

                # ---- phase C: i2t
                attn = wA.tile([128, NQ, 48], dt.float32, tag="attn")
                nc.vector.tensor_tensor(attn, lr3,
                                        rs.unsqueeze(2).to_broadcast([128, NQ, 48]),
                                        op=OP.mult)
                e = wA.tile([128, QC], dt.float32, tag="exp")
                nc.scalar.activation(e, attn.rearrange("p n r -> p (n r)"),
                                     AF.Exp, scale=LAM, bias=expb)
                eh = wA1.tile([128, QC], dt.bfloat16, tag="hi")
                nc.vector.tensor_copy(eh, e)
                el = wA1.tile([128, QC], dt.bfloat16, tag="lo")
                nc.vector.tensor_tensor(el, e, eh, op=OP.subtract)
                thr = rowp.tile([2, QC], dt.float32, tag="row")
                for j in range(JT2):
                    sl_ = slice(j * SL2, (j + 1) * SL2)
                    sps = psr.tile([2, SL2], dt.float32, tag="psr")
                    nc.tensor.matmul(sps, onesb, eh[:, sl_], start=True, stop=False)
                    nc.tensor.matmul(sps, onesb, el[:, sl_], start=False, stop=True)
                    nc.vector.tensor_scalar(thr[:, sl_], sps, invnw, None, op0=OP.mult)
                cmp = wA.tile([128, QC], dt.bfloat16, tag="cmp")
                for j in range(JT2):
                    sl_ = slice(j * SL2, (j + 1) * SL2)
                    bps = ps.tile([128, SL2], dt.float32, tag="ps")
                    nc.tensor.matmul(bps, onesf, thr[:, sl_], start=True, stop=True)
                    nc.vector.tensor_tensor(cmp[:, sl_], e[:, sl_], bps, op=OP.is_gt)
                e2b = wA.tile([128, QC], dt.bfloat16, tag="e2b")
                nc.vector.tensor_tensor(e2b, e, cmp, op=OP.mult)
                pp = wA.tile([128, QC], dt.bfloat16, tag="pp")
                nc.vector.tensor_tensor(pp, e2b, gb, op=OP.mult)
                w12row = rowp.tile([2, QC], dt.float32, tag="row")
                for j in range(JT2):
                    sl_ = slice(j * SL2, (j + 1) * SL2)
                    wps = psr.tile([2, SL2], dt.float32, tag="psr")
                    nc.tensor.matmul(wps, onesb, pp[:, sl_], start=True, stop=True)
                    nc.scalar.copy(w12row[:, sl_], wps)
                nc.sync.dma_start(out=w12_all[2 * p:2 * p + 2, :], in_=w12row)
                qq = wA.tile([128, QC], dt.bfloat16, tag="qq")
                for j in range(JT2):
                    sl_ = slice(j * SL2, (j + 1) * SL2)
                    hps = ps.tile([128, SL2], dt.float32, tag="ps")
                    nc.tensor.matmul(hps, k2bd, e2b[:, sl_], start=True, stop=True)
                    nc.vector.tensor_tensor(qq[:, sl_], e2b[:, sl_], hps, op=OP.mult)
                w2srow = rowp.tile([2, QC], dt.float32, tag="row")
                for j in range(JT2):
                    sl_ = slice(j * SL2, (j + 1) * SL2)
                    wps = psr.tile([2, SL2], dt.float32, tag="psr")
                    nc.tensor.matmul(wps, onesb, qq[:, sl_], start=True, stop=True)
                    nc.scalar.copy(w2srow[:, sl_], wps)
                nc.sync.dma_start(out=w2sq_all[2 * p:2 * p + 2, :], in_=w2srow)

                # ---- phase D: t2i
                attn2 = wA.tile([128, QC], dt.float32, tag="attn")
                for j in range(JT2):
                    sl_ = slice(j * SL2, (j + 1) * SL2)
                    bps = ps.tile([128, SL2], dt.float32, tag="ps")
                    nc.tensor.matmul(bps, onesf, rows[:, sl_], start=True, stop=True)
                    nc.vector.tensor_tensor(attn2[:, sl_], lr[:, sl_], bps, op=OP.mult)
                ee = wA.tile([128, QC], dt.float32, tag="exp")
                nc.scalar.activation(ee, attn2, AF.Exp, scale=LAM)
                ee3 = ee.rearrange("p (n r) -> p n r", r=48)
                sE = small.tile([128, NQ], dt.float32, tag="sE")
                nc.vector.tensor_reduce(sE, ee3, axis=AX.X, op=OP.add)
                th2 = small.tile([128, NQ], dt.float32, tag="th2")
                nc.vector.tensor_scalar(th2, sE, 1.0 / R, None, op0=OP.mult)
                cmp2 = wA.tile([128, QC], dt.bfloat16, tag="cmp")
                nc.vector.tensor_tensor(cmp2.rearrange("p (n r) -> p n r", r=48), ee3,
                                        th2.unsqueeze(2).to_broadcast([128, NQ, 48]),
                                        op=OP.is_gt)
                e2tb = wB.tile([128, QC], dt.bfloat16, tag="sq")
                nc.vector.tensor_tensor(e2tb, ee, cmp2, op=OP.mult)
                nump = wA.tile([128, QC], dt.bfloat16, tag="pp")
                nc.vector.tensor_tensor(nump, e2tb, gb, op=OP.mult)
                num = small.tile([128, NQ], dt.float32, tag="num")
                nc.vector.tensor_reduce(num, nump.rearrange("p (n r) -> p n r", r=48),
                                        axis=AX.X, op=OP.add)
                # W = a2T.T @ V, two images per matmul via block-diag V
                wq = wA.tile([128, QC], dt.bfloat16, tag="qq")
                for j in range(4):
                    wps = ps.tile([128, 384], dt.float32, tag="ps")
                    for k in range(4):
                        m = j * 4 + k          # BD pair index (2 images)
                        c0 = m * 96
                        tps = tpp.tile([96, 128], dt.bfloat16, tag="tp")
                        nc.tensor.transpose(tps, e2tb[:, c0:c0 + 96], ident)
                        a2t = small.tile([96, 128], dt.bfloat16, tag="a2t")
                        if k % 2 == 0:
                            nc.vector.tensor_copy(a2t, tps)
                        else:
                            nc.scalar.copy(a2t, tps)
                        nc.tensor.matmul(wps[:, k * 96:(k + 1) * 96], a2t,
                                         v_bd[:, m, :], start=True, stop=True)
                    sl_ = slice(j * 384, (j + 1) * 384)
                    nc.vector.tensor_tensor(wq[:, sl_], e2tb[:, sl_], wps, op=OP.mult)
                w2t = small.tile([128, NQ], dt.float32, tag="w2t")
                nc.vector.tensor_reduce(w2t, wq.rearrange("p (n r) -> p n r", r=48),
                                        axis=AX.X, op=OP.add)
                den2 = small.tile([128, NQ], dt.float32, tag="den2")
                nc.scalar.activation(den2, w2t, AF.Sqrt, bias=b20)
                nc.vector.reciprocal_approx_fast(out=den2, in_=den2)
                nc.vector.tensor_scalar(den2, den2, rw1t, None, op0=OP.mult)
                cos2 = small.tile([128, NQ], dt.float32, tag="cos2")
                nc.vector.tensor_tensor(cos2, num, den2, op=OP.mult)
                cos2m = small.tile([128, NQ], dt.bfloat16, tag="cos2m")
                nc.vector.tensor_scalar(cos2m, cos2, monw, None, op0=OP.mult)
                t2ips = psr.tile([2, NQ], dt.float32, tag="psr")
                nc.tensor.matmul(t2ips, onesb, cos2m, start=True, stop=True)
                t2irow = small.tile([2, NQ], dt.float32, tag="t2irow")
                nc.vector.tensor_copy(t2irow, t2ips)
                nc.sync.dma_start(out=t2i_o[2 * p:2 * p + 2, qt * NQ:(qt + 1) * NQ],
                                  in_=t2irow)

            # ---- quarter epilogue (in place on w2sq_all)
            nc.scalar.activation(w2sq_all, w2sq_all, AF.Sqrt, bias=b20[0:TPC])
            nc.vector.reciprocal_approx_fast(out=w2sq_all, in_=w2sq_all)
            nc.vector.tensor_tensor(w2sq_all, w2sq_all, rw1b_t, op=OP.mult)
            nc.vector.tensor_tensor(w2sq_all, w2sq_all, w12_all, op=OP.mult)
            i2t_sb = small.tile([TPC, NQ], dt.float32, tag="i2t_sb")
            nc.vector.tensor_reduce(i2t_sb,
                                    w2sq_all.rearrange("p (n r) -> p n r", r=48),
                                    axis=AX.X, op=OP.add)
            nc.vector.tensor_scalar(i2t_sb, i2t_sb, 1.0 / R, None, op0=OP.mult)
            nc.sync.dma_start(out=i2t_o[:, qt * NQ:(qt + 1) * NQ], in_=i2t_sb)

    return nc


def _l2n(x, ax):
    return x / (np.sqrt((x * x).sum(ax, keepdims=True)) + EPS)


def prep_inputs(local_img_query, local_img_value, local_text_key, local_text_value,
                text_length):
    img_q = np.ascontiguousarray(local_img_query, F32)
    img_v = np.ascontiguousarray(local_img_value, F32)
    txt_k = np.ascontiguousarray(local_text_key, F32)
    txt_v = np.ascontiguousarray(local_text_value, F32)
    tlen = np.asarray(text_length)

    qnT = _l2n(img_q, -1).reshape(NR, D).T.astype(F32)       # [D, NR]
    qhi = qnT.astype(BF)
    qlo = (qnT - qhi.astype(F32)).astype(BF)
    ivT = img_v.reshape(NR, D).T.astype(F32)
    w1 = np.sqrt((img_v.reshape(NR, D) ** 2).sum(-1)).astype(F32)

    ivb = ivT.astype(BF).astype(F32).T.reshape(N, R, D)
    V = np.einsum('nrd,nsd->nrs', ivb, ivb).astype(F32).astype(BF).astype(F32)
    vbd = np.zeros((QT, 96, NQ // 2 * 96), F32)
    for qt in range(QT):
        for m in range(NQ // 2):
            n0 = qt * NQ + 2 * m
            vbd[qt, :48, m * 96:m * 96 + 48] = V[n0]
            vbd[qt, 48:, m * 96 + 48:(m + 1) * 96] = V[n0 + 1]

    shared = {
        "qhi": np.ascontiguousarray(qhi.reshape(CH, 128, NR)),
        "qlo": np.ascontiguousarray(qlo.reshape(CH, 128, NR)),
        "ivt": np.ascontiguousarray(ivT.astype(BF).reshape(CH, 128, NR)),
        "rw1b": np.ascontiguousarray(
            np.broadcast_to((1.0 / np.maximum(w1, 1e-12)).astype(BF), (TPC, NR))),
        "vbd": vbd.astype(BF),
        "onesb": np.ascontiguousarray(
            np.stack([np.r_[np.ones(64), np.zeros(64)],
                      np.r_[np.zeros(64), np.ones(64)]], axis=1).astype(BF)),
        "onesf": np.ascontiguousarray(
            np.stack([np.r_[np.ones(64), np.zeros(64)],
                      np.r_[np.zeros(64), np.ones(64)]], axis=0).astype(F32)),
        "ident": np.eye(128, dtype=BF),
    }

    mask = (np.arange(Lw)[None, :] < tlen[:, None]).astype(F32)      # [T, Lw]
    tkm = _l2n(txt_k * mask[:, :, None], -1)
    tvm = txt_v * mask[:, :, None]

    in_maps = []
    for c in range(CORES):
        ts = slice(c * TPC, (c + 1) * TPC)
        tkhi = np.empty((PAIRS, CH, 128, 128), BF)
        tklo = np.empty((PAIRS, CH, 128, 128), BF)
        tvt = np.empty((PAIRS, CH, 128, 128), BF)
        k2bd = np.zeros((PAIRS, 128, 128), F32)
        rw1t = np.empty((PAIRS, 128, 1), F32)
        expb = np.empty((PAIRS, 128, 1), F32)
        monw = np.empty((PAIRS, 128, 1), F32)
        invnw = np.empty((PAIRS, 2, 1), F32)
        for p in range(PAIRS):
            t0, t1 = c * TPC + 2 * p, c * TPC + 2 * p + 1
            kT = np.concatenate([tkm[t0].T, tkm[t1].T], axis=1)      # [D, 128]
            vT = np.concatenate([tvm[t0].T, tvm[t1].T], axis=1)
            khi = kT.astype(BF)
            tkhi[p] = khi.reshape(CH, 128, 128)
            tklo[p] = (kT - khi.astype(F32)).astype(BF).reshape(CH, 128, 128)
            tvt[p] = vT.astype(BF).reshape(CH, 128, 128)
            vTf = vT.astype(BF).astype(F32)
            k2bd[p, :64, :64] = vTf[:, :64].T @ vTf[:, :64]
            k2bd[p, 64:, 64:] = vTf[:, 64:].T @ vTf[:, 64:]
            rw1t[p, :, 0] = 1.0 / np.maximum(np.sqrt(np.maximum(
                np.concatenate([(tvm[t0] ** 2).sum(-1), (tvm[t1] ** 2).sum(-1)]),
                0)), 1e-12)
            mk = np.concatenate([mask[t0], mask[t1]])
            expb[p, :, 0] = np.where(mk > 0, 0.0, -1000.0)
            monw[p, :, 0] = mk / np.concatenate(
                [np.full(64, tlen[t0]), np.full(64, tlen[t1])]).astype(F32)
            invnw[p, 0, 0] = 1.0 / F32(tlen[t0])
            invnw[p, 1, 0] = 1.0 / F32(tlen[t1])
        in_maps.append(dict(shared, tkhi=tkhi, tklo=tklo, tvt=tvt,
                            k2bd=k2bd.astype(BF), rw1t=rw1t, expb=expb, monw=monw,
                            invnw=invnw))
    return in_maps


_cache = {}


def get_nc():
    if "nc" not in _cache:
        nc = bacc.Bacc("TRN2", target_bir_lowering=False, debug=False,
                       num_devices=CORES)
        build_kernel(nc)
        nc.compile()
        _cache["nc"] = nc
    return _cache["nc"]


def kernel(local_img_query, local_img_value, local_text_key, local_text_value,
           text_length, _trace=False, _trace_kwargs=None):
    nc = get_nc()
    in_maps = prep_inputs(local_img_query, local_img_value, local_text_key,
                          local_text_value, text_length)
    res = run_bass_kernel_spmd(nc, in_maps, core_ids=list(range(CORES)),
                               trace=_trace, **(_trace_kwargs or {}))
    _cache["last_result"] = res
    i2t = np.concatenate([res.results[c]["i2t_loc"].T for c in range(CORES)], axis=1)
    t2i = np.concatenate([res.results[c]["t2i_loc"].T for c in range(CORES)], axis=1)
    return np.ascontiguousarray(i2t, F32), np.ascontiguousarray(t2i, F32)

